# revision 32
# baseline (speedup 1.0000x reference)
"""CenterLoss Trainium2 kernel.

Reference computes, for x[B,D], labels[B], centers[C,D]:
    distmat[b,c] = ||x_b||^2 + ||c_c||^2 - 2<x_b, c_c>
    dist = where(labels[b]==c, distmat, 0)
    loss = clip(dist, 1e-12, 1e12).sum() / B

Only one entry per row survives the mask: d_b = ||x_b - centers[labels_b]||^2.
The other C-1 zeros per row are clamped to 1e-12, contributing the constant
B*(C-1)*1e-12 to the sum.  The per-row clip is inactive for any real data
(d_b is a sum of 128 squared differences, ~256 in expectation; the bounds
are 1e-12 / 1e12), so:

    loss = ( sum_b d_b ) / B  +  (C-1)*1e-12

No [B,C] distmat needed: gather centers[labels] with SWDGE indirect DMAs
(8x128 rows, pipelined -- SWDGE descriptor generation runs at
~67.5ns/descriptor/GpSimd-subcore over 8 subcores, ~9us/core total, and is
the critical resource), then one fused custom-DVE op per 128-row chunk
computing (x-c)^2 * (1/B) with a free-dim accumulate (registered at build
time; replaces subtract + square + clip + reduce and saves the GpSimd
standard-library load the old cross-lane reduce needed).  The
cross-partition reduce runs on the otherwise-idle PE as
ones[128,1]^T @ dsum[128,8] -> PSUM [1,8], so the output DMA is a single
32-byte descriptor (a [128,x] SBUF source would need 128 descriptors and
queues behind the gather traffic).  Data-parallel over batch across 8
cores; the host sums the 8x8 partials.

Raw bacc, no Tile, no Block: engine programs are emitted straight into the
main basic block with manual semaphores.

Per-core layout: row r of the 1024-row shard lives at partition p = r//8,
free slot j = r%8 (x and label loads are contiguous DMAs; gather j fetches
rows {p*8+j} via per-partition offsets it[:, j]).
"""

import re

import numpy as np

B, C, D = 8192, 10000, 128
N_CORES = 8
RPC = B // N_CORES  # rows per core
P = 128
J = RPC // P  # free slots per partition

CLIP_LO = 1e-12
MASK_CONST = (C - 1) * CLIP_LO  # clamped masked-out zeros, after /B

_cache = {}


def _register_sqdiff_op():
    """Register a fused (Src0-Src1)^2*C2 DVE op with free-dim accumulate."""
    import concourse.dve_ops as dvo
    from concourse.dve_spec import C2, Spec, Src0, Src1, Zero, sq
    from operator import add

    if "sqdiff" in _cache:
        return _cache["sqdiff"]

    def _ref(in0, in1, s0, s1, imm2):
        b = (((in0.astype(np.float32) - in1) ** 2) * imm2).astype(np.float32)
        return b, b.reshape(b.shape[0], -1).sum(axis=-1, keepdims=True)

    spec = Spec(body=sq(Src0 - Src1) * C2, accum=add, accum_init=Zero, reference=_ref)
    op = dvo.DveOp("SQDIFF_ACC_CL", spec, subdim=False, uops_sha={})
    dvo._SUB_OPCODE_FOR_NAME[op.name] = dvo._CUSTOM_DVE_ROW_BASE + len(dvo.OPS)
    dvo.OPS.append(op)
    dvo.CUSTOM_DVE_SPECS[op.name] = spec
    # pin the uops sha (computed, not hand-maintained, for this session)
    for ver in ("v3", "v4"):
        try:
            op.compile(ver)
        except ValueError as e:
            m = re.search(r'="([0-9a-f]+)"', str(e))
            op.uops_sha[ver] = m.group(1)
            op.compile(ver)
    _cache["sqdiff"] = op
    return op


def _build():
    from contextlib import ExitStack

    import concourse.bacc as bacc
    import concourse.bass as bass
    import concourse.mybir as mybir

    f32 = mybir.dt.float32
    i32 = mybir.dt.int32
    sqdiff = _register_sqdiff_op()

    class _FastBacc(bacc.Bacc):
        # the init-time all-engine barrier only guards the const-ap
        # memsets, which only the final PE reduce reads (ones vector,
        # long after every engine passed the entry barrier) — skip it
        def all_engine_barrier(self, **kw):
            return

    # PE preamble's config write is already performed by the NEFF scaffold
    pe_preamble = bass.BassTensorEngine.preamble
    bass.BassTensorEngine.preamble = lambda self: None
    try:
        nc = _FastBacc("TRN2", target_bir_lowering=False, debug=False)
    finally:
        bass.BassTensorEngine.preamble = pe_preamble

    x_d = nc.dram_tensor("x", [RPC, D], f32, kind="ExternalInput")
    lab_d = nc.dram_tensor("labels", [P, J], i32, kind="ExternalInput")
    cen_d = nc.dram_tensor("centers", [C, D], f32, kind="ExternalInput")
    out_d = nc.dram_tensor("out", [1, 1], f32, kind="ExternalOutput")

    ones = nc.const_aps.aps[(f32, 1.0)]  # [128, 1] memset at init

    with (
        ExitStack() as ctx,
        nc.sbuf_tensor("xt", [P, J, D], f32) as xt,
        nc.sbuf_tensor("ct", [P, J, D], f32) as ct,
        nc.sbuf_tensor("sq2", [P, J, D], f32) as sq2,
        nc.sbuf_tensor("it", [P, J], i32) as it,
        nc.sbuf_tensor("dsum", [P, J], f32) as dsum,
        nc.sbuf_tensor("res_sb", [1, J], f32) as res_sb,
        nc.sbuf_tensor("res1", [1, 1], f32) as res1,
        nc.psum_tensor("res", [1, J], f32) as res,
        nc.semaphore("s_idx") as s_idx,
        nc.semaphore("s_x") as s_x,
        nc.semaphore("s_v") as s_v,
        nc.semaphore("s_mm") as s_mm,
        nc.semaphore("s_out") as s_out,
    ):
        s_g = [ctx.enter_context(nc.semaphore(f"s_g{j}")) for j in range(J)]  # noqa: ANT232

        # ---- Sync: idx DMA strictly first (its receipt gates the gathers),
        # then x with contiguous 4KB-per-partition descriptors
        nc.sync.dma_start(out=it[:], in_=lab_d[:, :]).then_inc(s_idx, 16)
        x_ap = x_d[:, :].rearrange("(p j) d -> p (j d)", p=P)
        nc.sync.dma_start(
            out=xt[:].rearrange("p j d -> p (j d)"), in_=x_ap
        ).then_inc(s_x, 16)
        nc.sync.wait_ge(s_out, 16)

        # ---- GpSimd: per-slot indirect gathers (multi-slot offset APs
        # mis-pair offsets with destinations and defer descriptor expansion;
        # per-slot [128,1] gathers are the fast, correct SWDGE form)
        nc.gpsimd.wait_ge(s_idx, 16)
        for j in range(J):
            nc.gpsimd.indirect_dma_start(
                out=ct[:, j, :],
                out_offset=None,
                in_=cen_d[:, :],
                in_offset=bass.IndirectOffsetOnAxis(ap=it[:, j : j + 1], axis=0),
            ).then_inc(s_g[j], 16)
        # keep GpSimd's SEQ parked here so the Bass teardown (sem resets +
        # dma_reset drain) can't dispatch during the gathers and slow their
        # descriptor generation (~+220ns per gather observed); s_v (not
        # s_out) so the teardown overlaps the PE/scalar/out-DMA tail
        nc.gpsimd.wait_ge(s_v, 1)

        # ---- Vector: per-chunk fused (x-c)^2/B with free-dim accumulate
        nc.vector.wait_ge(s_x, 16)
        for j in range(J):
            nc.vector.wait_ge(s_g[j], 16)
            nc.vector._custom_dve(
                sqdiff,
                out=sq2[:, j, :],
                in0=xt[:, j, :],
                in1=ct[:, j, :],
                imm2=1.0 / B,
                accum_out=dsum[:, j : j + 1],
            )
        nc.vector.drain().then_inc(s_v, 1)

        # ---- PE: cross-partition reduce, ones^T @ dsum -> [1, J] in PSUM
        nc.tensor.wait_ge(s_v, 1)
        nc.tensor.matmul(out=res[:], lhsT=ones, rhs=dsum[:]).then_inc(s_mm, 1)

        # ---- Scalar: PSUM -> SBUF copy with free-dim accumulate -> [1, 1]
        # (DMA cannot read PSUM, and a [1,1] result keeps the output DMA a
        # single descriptor with one semaphore write)
        nc.scalar.wait_ge(s_mm, 1)
        nc.scalar.activation(
            out=res_sb[:],
            in_=res[:],
            func=mybir.ActivationFunctionType.Copy,
            accum_out=res1[:],
        ).then_inc(s_mm, 1)
        # Scalar is a HWDGE engine: issue the output DMA from its own queue
        # right after the accumulate, no cross-engine hop
        nc.scalar.wait_ge(s_mm, 2)
        nc.scalar.dma_start(out=out_d[:, :], in_=res1[:]).then_inc(s_out, 16)
        nc.scalar.wait_ge(s_out, 16)

    nc.compile()
    return nc


def _get_nc():
    if "nc" not in _cache:
        _cache["nc"] = _build()
    return _cache["nc"]


def _make_in_maps(x, labels, centers):
    x = np.ascontiguousarray(np.asarray(x, dtype=np.float32))
    labels = np.asarray(labels).astype(np.int32)
    centers = np.ascontiguousarray(np.asarray(centers, dtype=np.float32))
    in_maps = []
    for i in range(N_CORES):
        sl = slice(i * RPC, (i + 1) * RPC)
        in_maps.append(
            {
                "x": x[sl],
                "labels": np.ascontiguousarray(labels[sl].reshape(P, J)),
                "centers": centers,
            }
        )
    return in_maps


def _run(in_maps, trace=False, **kwargs):
    from concourse.bass_utils import run_bass_kernel_spmd

    nc = _get_nc()
    return run_bass_kernel_spmd(
        nc, in_maps, core_ids=list(range(N_CORES)), trace=trace, **kwargs
    )


def kernel(x, labels, centers):
    res = _run(_make_in_maps(x, labels, centers))
    total = np.float64(0.0)
    for r in res.results:
        total += r["out"].astype(np.float64).sum()
    return np.asarray(np.float32(total) + np.float32(MASK_CONST), dtype=np.float32)


# revision 34
# speedup vs baseline: 1.0087x; 1.0087x over previous
"""CenterLoss Trainium2 kernel.

Reference computes, for x[B,D], labels[B], centers[C,D]:
    distmat[b,c] = ||x_b||^2 + ||c_c||^2 - 2<x_b, c_c>
    dist = where(labels[b]==c, distmat, 0)
    loss = clip(dist, 1e-12, 1e12).sum() / B

Only one entry per row survives the mask: d_b = ||x_b - centers[labels_b]||^2.
The other C-1 zeros per row are clamped to 1e-12, contributing the constant
B*(C-1)*1e-12 to the sum.  The per-row clip is inactive for any real data
(d_b is a sum of 128 squared differences, ~256 in expectation; the bounds
are 1e-12 / 1e12), so:

    loss = ( sum_b d_b ) / B  +  (C-1)*1e-12

No [B,C] distmat needed: gather centers[labels] with SWDGE indirect DMAs
(8x128 rows, pipelined -- SWDGE descriptor generation runs at
~67.5ns/descriptor/GpSimd-subcore over 8 subcores, ~9us/core total, and is
the critical resource), then one fused custom-DVE op per 128-row chunk
computing (x-c)^2 * (1/B) with a free-dim accumulate (registered at build
time; replaces subtract + square + clip + reduce and saves the GpSimd
standard-library load the old cross-lane reduce needed).  The
cross-partition reduce runs on the otherwise-idle PE as
ones[128,1]^T @ dsum[128,8] -> PSUM [1,8], so the output DMA is a single
32-byte descriptor (a [128,x] SBUF source would need 128 descriptors and
queues behind the gather traffic).  Data-parallel over batch across 8
cores; the host sums the 8x8 partials.

Raw bacc, no Tile, no Block: engine programs are emitted straight into the
main basic block with manual semaphores.

Per-core layout: row r of the 1024-row shard lives at partition p = r//8,
free slot j = r%8 (x and label loads are contiguous DMAs; gather j fetches
rows {p*8+j} via per-partition offsets it[:, j]).
"""

import re

import numpy as np

B, C, D = 8192, 10000, 128
N_CORES = 8
RPC = B // N_CORES  # rows per core
P = 128
J = RPC // P  # free slots per partition

CLIP_LO = 1e-12
MASK_CONST = (C - 1) * CLIP_LO  # clamped masked-out zeros, after /B

_cache = {}


def _register_sqdiff_op():
    """Register a fused (Src0-Src1)^2*C2 DVE op with free-dim accumulate."""
    import concourse.dve_ops as dvo
    from concourse.dve_spec import C2, Spec, Src0, Src1, Zero, sq
    from operator import add

    if "sqdiff" in _cache:
        return _cache["sqdiff"]

    def _ref(in0, in1, s0, s1, imm2):
        b = (((in0.astype(np.float32) - in1) ** 2) * imm2).astype(np.float32)
        return b, b.reshape(b.shape[0], -1).sum(axis=-1, keepdims=True)

    spec = Spec(body=sq(Src0 - Src1) * C2, accum=add, accum_init=Zero, reference=_ref)
    op = dvo.DveOp("SQDIFF_ACC_CL", spec, subdim=False, uops_sha={})
    dvo._SUB_OPCODE_FOR_NAME[op.name] = dvo._CUSTOM_DVE_ROW_BASE + len(dvo.OPS)
    dvo.OPS.append(op)
    dvo.CUSTOM_DVE_SPECS[op.name] = spec
    # pin the uops sha (computed, not hand-maintained, for this session)
    for ver in ("v3", "v4"):
        try:
            op.compile(ver)
        except ValueError as e:
            m = re.search(r'="([0-9a-f]+)"', str(e))
            op.uops_sha[ver] = m.group(1)
            op.compile(ver)
    _cache["sqdiff"] = op
    return op


def _build():
    from contextlib import ExitStack

    import concourse.bacc as bacc
    import concourse.bass as bass
    import concourse.mybir as mybir

    f32 = mybir.dt.float32
    i32 = mybir.dt.int32
    sqdiff = _register_sqdiff_op()

    class _FastBacc(bacc.Bacc):
        # the init-time all-engine barrier only guards the const-ap
        # memsets, which only the final PE reduce reads (ones vector,
        # long after every engine passed the entry barrier) — skip it
        def all_engine_barrier(self, **kw):
            return

    # PE preamble's config write is already performed by the NEFF scaffold
    pe_preamble = bass.BassTensorEngine.preamble
    bass.BassTensorEngine.preamble = lambda self: None
    try:
        nc = _FastBacc("TRN2", target_bir_lowering=False, debug=False)
    finally:
        bass.BassTensorEngine.preamble = pe_preamble

    x_d = nc.dram_tensor("x", [RPC, D], f32, kind="ExternalInput")
    lab_d = nc.dram_tensor("labels", [P, J], i32, kind="ExternalInput")
    cen_d = nc.dram_tensor("centers", [C, D], f32, kind="ExternalInput")
    out_d = nc.dram_tensor("out", [1, 1], f32, kind="ExternalOutput")

    ones = nc.const_aps.aps[(f32, 1.0)]  # [128, 1] memset at init

    with (
        ExitStack() as ctx,
        nc.sbuf_tensor("xt", [P, J, D], f32) as xt,
        nc.sbuf_tensor("ct", [P, J, D], f32) as ct,
        nc.sbuf_tensor("sq2", [P, J, D], f32) as sq2,
        nc.sbuf_tensor("it", [P, J], i32) as it,
        nc.sbuf_tensor("dsum", [P, J], f32) as dsum,
        nc.sbuf_tensor("res_sb", [1, J], f32) as res_sb,
        nc.sbuf_tensor("res1", [1, 1], f32) as res1,
        nc.psum_tensor("res", [1, J], f32) as res,
        nc.semaphore("s_idx") as s_idx,
        nc.semaphore("s_x") as s_x,
        nc.semaphore("s_v") as s_v,
        nc.semaphore("s_mm") as s_mm,
        nc.semaphore("s_out") as s_out,
    ):
        s_g = [ctx.enter_context(nc.semaphore(f"s_g{j}")) for j in range(J)]  # noqa: ANT232

        # ---- Sync: idx DMA strictly first (its receipt gates the gathers),
        # then x with contiguous 4KB-per-partition descriptors
        nc.sync.dma_start(out=it[:], in_=lab_d[:, :]).then_inc(s_idx, 16)
        x_ap = x_d[:, :].rearrange("(p j) d -> p (j d)", p=P)
        nc.sync.dma_start(
            out=xt[:].rearrange("p j d -> p (j d)"), in_=x_ap
        ).then_inc(s_x, 16)
        nc.sync.wait_ge(s_mm, 2)
        nc.sync.dma_start(out=out_d[:, :], in_=res1[:]).then_inc(s_out, 16)
        nc.sync.wait_ge(s_out, 16)

        # ---- GpSimd: per-slot indirect gathers (multi-slot offset APs
        # mis-pair offsets with destinations and defer descriptor expansion;
        # per-slot [128,1] gathers are the fast, correct SWDGE form)
        nc.gpsimd.wait_ge(s_idx, 16)
        for j in range(J):
            nc.gpsimd.indirect_dma_start(
                out=ct[:, j, :],
                out_offset=None,
                in_=cen_d[:, :],
                in_offset=bass.IndirectOffsetOnAxis(ap=it[:, j : j + 1], axis=0),
            ).then_inc(s_g[j], 16)
        # keep GpSimd's SEQ parked here so the Bass teardown (sem resets +
        # dma_reset drain) can't dispatch during the gathers and slow their
        # descriptor generation (~+220ns per gather observed); s_v (not
        # s_out) so the teardown overlaps the PE/scalar/out-DMA tail
        nc.gpsimd.wait_ge(s_v, 1)

        # ---- Vector: per-chunk fused (x-c)^2/B with free-dim accumulate
        nc.vector.wait_ge(s_x, 16)
        for j in range(J):
            nc.vector.wait_ge(s_g[j], 16)
            nc.vector._custom_dve(
                sqdiff,
                out=sq2[:, j, :],
                in0=xt[:, j, :],
                in1=ct[:, j, :],
                imm2=1.0 / B,
                accum_out=dsum[:, j : j + 1],
            )
        nc.vector.drain().then_inc(s_v, 1)

        # ---- PE: cross-partition reduce, ones^T @ dsum -> [1, J] in PSUM
        nc.tensor.wait_ge(s_v, 1)
        nc.tensor.matmul(out=res[:], lhsT=ones, rhs=dsum[:]).then_inc(s_mm, 1)

        # ---- Scalar: PSUM -> SBUF copy with free-dim accumulate -> [1, 1]
        # (DMA cannot read PSUM, and a [1,1] result keeps the output DMA a
        # single descriptor with one semaphore write)
        nc.scalar.wait_ge(s_mm, 1)
        nc.scalar.activation(
            out=res_sb[:],
            in_=res[:],
            func=mybir.ActivationFunctionType.Copy,
            accum_out=res1[:],
        ).then_inc(s_mm, 1)

    nc.compile()
    return nc


def _get_nc():
    if "nc" not in _cache:
        _cache["nc"] = _build()
    return _cache["nc"]


def _make_in_maps(x, labels, centers):
    x = np.ascontiguousarray(np.asarray(x, dtype=np.float32))
    labels = np.asarray(labels).astype(np.int32)
    centers = np.ascontiguousarray(np.asarray(centers, dtype=np.float32))
    in_maps = []
    for i in range(N_CORES):
        sl = slice(i * RPC, (i + 1) * RPC)
        in_maps.append(
            {
                "x": x[sl],
                "labels": np.ascontiguousarray(labels[sl].reshape(P, J)),
                "centers": centers,
            }
        )
    return in_maps


def _run(in_maps, trace=False, **kwargs):
    from concourse.bass_utils import run_bass_kernel_spmd

    nc = _get_nc()
    return run_bass_kernel_spmd(
        nc, in_maps, core_ids=list(range(N_CORES)), trace=trace, **kwargs
    )


def kernel(x, labels, centers):
    res = _run(_make_in_maps(x, labels, centers))
    total = np.float64(0.0)
    for r in res.results:
        total += r["out"].astype(np.float64).sum()
    return np.asarray(np.float32(total) + np.float32(MASK_CONST), dtype=np.float32)
